# revision 22
# baseline (speedup 1.0000x reference)
"""Trainium2 Bass kernel for nn_BoundaryConsistencyLoss (v3).

Math
----
Inputs seg/gt are binary {0,1} images [64, 512, 512].  For binary x the
per-class boundary (dilation - erosion with -inf padding) is the same for
both classes: b[i,j] = 1 iff the in-bounds 3x3 window at (i,j) is
non-constant.  The loss reduces to 4 * mean(xor(L(b_s), L(b_g))) with L the
row-then-column line-removal operator; the xor-count identity lets the host
finish from per-sample masked column sums (all exact small integers).

Let wsum = 3x3 replicate-pad window sum (total weight 9 everywhere) and
u = 9*x - wsum in [-8, 8].  Then b = (u != 0) -- a ONE-SIDED test, so the
fused DVE op  tensor_scalar(not_equal, accum=rowsum)  produces b and its
row sums in a single pass, and ACT can alternatively produce it with the
exact pair Square -> Sign(+accum).  u is built entirely on the PE with fp8
DoubleRow matmuls (K=256): 3 shifted band matmuls (C_center = 9*delta -
vband folds the 9x term into the center tap) + 2 single-column edge fixups.

Device per (sample, tensor): rows 0..507 as 4 blocks of 127 rows; the last
4 rows (508..511) are finished on the host (<1% of pixels).  Per sample the
device emits masked column sums cs_s, cs_g, P = sum_r m_s m_g b_s b_g via
fp8 DoubleRow matmuls with the row-mask vectors as stationary operands,
accumulated in PSUM and DMA'd straight to DRAM (no evacuation pass).

Sharding: pure data parallel over batch, 8 samples per NeuronCore.
"""

from contextlib import ExitStack

import ml_dtypes
import numpy as np

import concourse.bacc as bacc
import concourse.mybir as mybir
import concourse.tile as tile
from concourse import bass_utils

# ---------------------------------------------------------------- config
B, H, W = 64, 512, 512
N_CORES = 8
BPC = B // N_CORES          # samples per core
ST = 2 * BPC                # sample-tensors per core
LINE_T = 300.0

NP_FP8 = ml_dtypes.float8_e4m3fn

# engine split knobs
ACT_MOD, ACT_LT = 16, 7     # threshold unit -> ACT if (unit % MOD) < LT
GPS_MOD, GPS_LT = 8, 3      # q unit -> GpSimd if (qunit % MOD) < LT

# row-block geometry: pair P covers input rows [253P, 253P+255] as two
# DoubleRow k-tiles; block (P, blk) outputs rows 254P + 127*blk .. +126.
PAIR_IN0 = (0, 253)
NBLK = 4                    # blocks per sample-tensor (rows 0..507)
REM0 = 508                  # host-handled remainder rows REM0..511


def _vband(g: int, in_row: int) -> int:
    """Vertical 3-tap replicate-pad weight of input row `in_row` for output
    row g (rows 0..511; only the top pad matters, bottom rows are host's)."""
    w = 0
    for d in (-1, 0, 1):
        t = g + d
        if t < 0:
            t = 0
        if t == in_row:
            w += 1
    return w


def _build_bands() -> np.ndarray:
    """bands[P*4 + blk*2 + kind] = [128, 2*127] fp8 lhsT (kind 0=center with
    the 9*delta fold, 1=side)."""
    out = np.zeros((8, 128, 2, 128), np.float32)
    for P in range(2):
        base = PAIR_IN0[P]
        for blk in range(2):
            for m in range(127):
                g = 254 * P + 127 * blk + m
                for k in range(2):
                    for p in range(128):
                        in_row = base + 128 * k + p
                        v = _vband(g, in_row)
                        out[P * 4 + blk * 2 + 0, p, k, m] = (
                            9.0 * (in_row == g) - v
                        )
                        out[P * 4 + blk * 2 + 1, p, k, m] = -v
    return out.reshape(8, 128, 256).astype(NP_FP8)


def _build_module(bpc: int = BPC):
    nc = bacc.Bacc("TRN2")
    f32 = mybir.dt.float32
    bf16 = mybir.dt.bfloat16
    fp8 = mybir.dt.float8e4
    Alu = mybir.AluOpType
    Act = mybir.ActivationFunctionType
    DR = mybir.MatmulPerfMode.DoubleRow

    st = 2 * bpc
    xdev = nc.dram_tensor("xdev", [st, 128, 4 * W], fp8, kind="ExternalInput")
    bands = nc.dram_tensor("bands", [8, 128, 256], fp8, kind="ExternalInput")
    outd = nc.dram_tensor("out", [6, 97, W], f32, kind="ExternalOutput")

    with tile.TileContext(nc) as tc, ExitStack() as ctx:
        const = ctx.enter_context(tc.tile_pool(name="const", bufs=1))
        xp = ctx.enter_context(tc.tile_pool(name="xp", bufs=4))
        bpools = [
            ctx.enter_context(tc.tile_pool(name=f"b{i}", bufs=2)) for i in range(4)
        ]
        qpools = [
            ctx.enter_context(tc.tile_pool(name=f"q{i}", bufs=2)) for i in range(2)
        ]
        sqp = ctx.enter_context(tc.tile_pool(name="sq", bufs=3))
        rsp = ctx.enter_context(tc.tile_pool(name="rs", bufs=4))
        mp = ctx.enter_context(tc.tile_pool(name="mp", bufs=6))
        evp = ctx.enter_context(tc.tile_pool(name="evp", bufs=6))
        pv = ctx.enter_context(tc.tile_pool(name="pv", bufs=2, space="PSUM"))
        pcs = ctx.enter_context(tc.tile_pool(name="pcs", bufs=1, space="PSUM"))

        band_t = []
        for i in range(8):
            bt = const.tile([128, 2, 128], fp8, name=f"bt{i}", tag=f"bt{i}")
            nc.sync.dma_start(out=bt[:], in_=bands[i])
            band_t.append(bt)

        used_banks = sorted({2 * qi + s // 4 for qi in range(3) for s in range(bpc)})
        cs_tiles = {
            i: pcs.tile([128, W], f32, name=f"cs{i}", tag=f"cs{i}")
            for i in used_banks
        }
        for i in used_banks:
            nc.vector.memset(cs_tiles[i][:], 0.0)

        for s in range(bpc):
            Xt = []
            for t in range(2):
                X = xp.tile([128, 4, W], fp8, tag=f"x{t}")
                nc.sync.dma_start(out=X[:], in_=xdev[2 * s + t])
                Xt.append(X)

            rst = [
                rsp.tile([128, 4], f32, name=f"rs{t}", tag=f"rs{t}")
                for t in range(2)
            ]
            btiles = {}
            qtiles = []
            for P in range(2):
                for t in range(2):
                    btile = bpools[2 * t + P].tile([128, 2, W], fp8, tag="b")
                    btiles[(t, P)] = btile
                    for blk in range(2):
                        u = pv.tile([128, W], f32, tag="u")
                        Cc = band_t[P * 4 + blk * 2 + 0]
                        Cs = band_t[P * 4 + blk * 2 + 1]
                        rhs = Xt[t][:, 2 * P : 2 * P + 2, :]
                        nc.tensor.matmul(u[:, 0:W], Cc[:], rhs[:, :, 0:W],
                                         start=True, stop=False, perf_mode=DR)
                        nc.tensor.matmul(u[:, 0 : W - 1], Cs[:],
                                         rhs[:, :, 1:W],
                                         start=False, stop=False, perf_mode=DR)
                        nc.tensor.matmul(u[:, 1:W], Cs[:],
                                         rhs[:, :, 0 : W - 1],
                                         start=False, stop=False, perf_mode=DR)
                        nc.tensor.matmul(u[:, 0:1], Cs[:], rhs[:, :, 0:1],
                                         start=False, stop=False, perf_mode=DR)
                        nc.tensor.matmul(u[:, W - 1 : W], Cs[:],
                                         rhs[:, :, W - 1 : W],
                                         start=False, stop=True, perf_mode=DR)

                        unit = (2 * s + t) * 4 + P * 2 + blk
                        col = 2 * P + blk
                        bout = btile[:, blk, :]
                        racc = rst[t][:, col : col + 1]
                        if unit % ACT_MOD < ACT_LT:
                            sq = sqp.tile([128, W], bf16, tag="sq")
                            nc.scalar.activation(sq[:], u[:], Act.Square)
                            nc.scalar.activation(bout, sq[:], Act.Sign,
                                                 accum_out=racc)
                        else:
                            nc.vector.tensor_scalar(bout, u[:], 0.0, None,
                                                    Alu.not_equal, Alu.add,
                                                    accum_out=racc)
                # q for this pair
                qtile = qpools[P].tile([128, 2, W], fp8, tag="q")
                qtiles.append(qtile)
                qunit = 2 * s + P
                eng = nc.gpsimd if qunit % GPS_MOD < GPS_LT else nc.vector
                eng.tensor_tensor(qtile[:, :, :], btiles[(0, P)][:, :, :],
                                  btiles[(1, P)][:, :, :], Alu.mult)

            # row masks (tiny) on gpsimd
            ms = []
            for t in range(2):
                m = mp.tile([128, 4], fp8, tag=f"m{t}")
                nc.gpsimd.tensor_scalar(m[:], rst[t][:], LINE_T, None, Alu.is_lt)
                ms.append(m)
            mq = mp.tile([128, 4], fp8, tag="mq")
            nc.gpsimd.tensor_tensor(mq[:], ms[0][:], ms[1][:], Alu.mult)

            # masked column sums: 3 quantities x 2 pairs, DoubleRow M=1
            part = 32 * (s % 4)
            g4 = s // 4
            plan = [
                (ms[0], [btiles[(0, 0)], btiles[(0, 1)]]),
                (ms[1], [btiles[(1, 0)], btiles[(1, 1)]]),
                (mq, qtiles),
            ]
            for qi, (mask, rhs_pair) in enumerate(plan):
                cst = cs_tiles[2 * qi + g4]
                for P in range(2):
                    for blk in range(2):
                        col = 2 * P + blk
                        nc.tensor.matmul(cst[part : part + 1, :],
                                         mask[:, col : col + 1],
                                         rhs_pair[P][:, blk, :],
                                         start=(col == 0), stop=(col == 3),
                                         tile_position=(0, part))

        # evacuate the used partition rows of each bank, then DMA out
        for bank in used_banks:
            ev = evp.tile([97, W], f32, name=f"ev{bank}", tag=f"ev{bank}")
            if bank % 2 == 0:
                nc.scalar.copy(ev[:], cs_tiles[bank][0:97, :])
            else:
                nc.vector.tensor_copy(ev[:], cs_tiles[bank][0:97, :])
            nc.sync.dma_start(out=outd[bank], in_=ev[:])

    nc.compile()
    return nc


# ---------------------------------------------------------------- host side

_ROWMAP = np.concatenate(
    [np.arange(0, 128), np.arange(128, 256), np.arange(253, 381),
     np.arange(381, 509)]
)

_CACHE: dict = {}


def _get_module():
    if "nc" not in _CACHE:
        _CACHE["nc"] = _build_module()
        _CACHE["bands"] = _build_bands()
    return _CACHE["nc"], _CACHE["bands"]


def _pack_core(seg8: np.ndarray, gt8: np.ndarray) -> np.ndarray:
    """[8,512,512] f32 x2 -> xdev [16, 128, 2048] fp8 (ktile layout)."""
    xdev = np.empty((ST, 128, 4 * W), NP_FP8)
    for s in range(BPC):
        for t, arr in ((0, seg8), (1, gt8)):
            v = arr[s][_ROWMAP]                       # [512, 512]
            v = v.reshape(4, 128, W).transpose(1, 0, 2).reshape(128, 4 * W)
            xdev[2 * s + t] = v.astype(NP_FP8)
    return xdev


def _host_rem(x: np.ndarray):
    """Boundary rows REM0..511 for all samples of one tensor.
    x: [B_, H, W] float {0,1}.  Returns (b [B_,4,W] f64, m [B_,4] f64)."""
    sub = x[:, REM0 - 1 : H, :].astype(np.float64)    # rows 507..511
    hp = np.pad(sub, ((0, 0), (0, 0), (1, 1)), mode="edge")
    h3 = hp[:, :, 0:W] + hp[:, :, 1 : W + 1] + hp[:, :, 2 : W + 2]
    w = np.empty((x.shape[0], 4, W))
    for i in range(4):
        # out row 508+i uses rows {507+i, 508+i, min(509+i, 511)} (replicate)
        lo, mid, hi = i, i + 1, min(i + 2, 4)
        w[:, i] = h3[:, lo] + h3[:, mid] + h3[:, hi]
    b = ((w > 0.5) & (w < 8.5)).astype(np.float64)
    m = (b.sum(axis=2) < LINE_T).astype(np.float64)
    return b, m


def _finish(outs: list[np.ndarray], seg: np.ndarray, gt: np.ndarray) -> np.ndarray:
    """outs: per-core 'out' arrays [6, 128, 512] f32."""
    bs_rem, ms_rem = _host_rem(seg)
    bg_rem, mg_rem = _host_rem(gt)
    total = 0.0
    for c, res in enumerate(outs):
        for s in range(BPC):
            part = 32 * (s % 4)
            g4 = s // 4
            gs = c * BPC + s
            cs_s = res[0 + g4, part].astype(np.float64)
            cs_g = res[2 + g4, part].astype(np.float64)
            Pv = res[4 + g4, part].astype(np.float64)
            cs_s = cs_s + (ms_rem[gs, :, None] * bs_rem[gs]).sum(axis=0)
            cs_g = cs_g + (mg_rem[gs, :, None] * bg_rem[gs]).sum(axis=0)
            Pv = Pv + (
                (ms_rem[gs] * mg_rem[gs])[:, None] * bs_rem[gs] * bg_rem[gs]
            ).sum(axis=0)
            ok_s = (cs_s < LINE_T).astype(np.float64)
            ok_g = (cs_g < LINE_T).astype(np.float64)
            total += float(
                np.sum(cs_s * ok_s) + np.sum(cs_g * ok_g)
                - 2.0 * np.sum(Pv * ok_s * ok_g)
            )
    return np.asarray(np.float32(4.0 * total / float(B * H * W)))


def make_in_maps(seg: np.ndarray, gt: np.ndarray, bands: np.ndarray):
    seg = np.ascontiguousarray(seg, dtype=np.float32)
    gt = np.ascontiguousarray(gt, dtype=np.float32)
    return [
        {
            "xdev": _pack_core(seg[c * BPC : (c + 1) * BPC],
                               gt[c * BPC : (c + 1) * BPC]),
            "bands": bands,
        }
        for c in range(N_CORES)
    ]


def kernel(seg: np.ndarray, gt: np.ndarray) -> np.ndarray:
    nc, bands = _get_module()
    in_maps = make_in_maps(seg, gt, bands)
    r = bass_utils.run_bass_kernel_spmd(nc, in_maps, core_ids=list(range(N_CORES)))
    return _finish([r.results[c]["out"] for c in range(N_CORES)], seg, gt)


# revision 25
# speedup vs baseline: 1.1536x; 1.1536x over previous
"""Trainium2 Bass kernel for nn_BoundaryConsistencyLoss (v3).

Math
----
Inputs seg/gt are binary {0,1} images [64, 512, 512].  For binary x the
per-class boundary (dilation - erosion with -inf padding) is the same for
both classes: b[i,j] = 1 iff the in-bounds 3x3 window at (i,j) is
non-constant.  The loss reduces to 4 * mean(xor(L(b_s), L(b_g))) with L the
row-then-column line-removal operator; the xor-count identity lets the host
finish from per-sample masked column sums (all exact small integers).

Let wsum = 3x3 replicate-pad window sum (total weight 9 everywhere) and
u = 9*x - wsum in [-8, 8].  Then b = (u != 0) -- a ONE-SIDED test, so the
fused DVE op  tensor_scalar(not_equal, accum=rowsum)  produces b and its
row sums in a single pass, and ACT can alternatively produce it with the
exact pair Square -> Sign(+accum).  u is built entirely on the PE with fp8
DoubleRow matmuls (K=256): 3 shifted band matmuls (C_center = 9*delta -
vband folds the 9x term into the center tap) + 2 single-column edge fixups.

Device per (sample, tensor): rows 0..507 as 4 blocks of 127 rows; the last
4 rows (508..511) are finished on the host (<1% of pixels).  Per sample the
device emits masked column sums cs_s, cs_g, P = sum_r m_s m_g b_s b_g via
fp8 DoubleRow matmuls with the row-mask vectors as stationary operands,
accumulated in PSUM and DMA'd straight to DRAM (no evacuation pass).

Sharding: pure data parallel over batch, 8 samples per NeuronCore.
"""

from contextlib import ExitStack

import ml_dtypes
import numpy as np

import concourse.bacc as bacc
import concourse.mybir as mybir
import concourse.tile as tile
from concourse import bass_utils

# ---------------------------------------------------------------- config
B, H, W = 64, 512, 512
WP = W + 2              # width with replicate-padded edge columns
N_CORES = 8
BPC = B // N_CORES          # samples per core
ST = 2 * BPC                # sample-tensors per core
LINE_T = 300.0

NP_FP8 = ml_dtypes.float8_e4m3fn

# engine split knobs
ACT_MOD, ACT_LT = 16, 6     # threshold unit -> ACT if (unit % MOD) < LT
GPS_MOD, GPS_LT = 8, 2      # q unit -> GpSimd if (qunit % MOD) < LT

# row-block geometry: pair P covers input rows [253P, 253P+255] as two
# DoubleRow k-tiles; block (P, blk) outputs rows 254P + 127*blk .. +126.
PAIR_IN0 = (0, 253)
NBLK = 4                    # blocks per sample-tensor (rows 0..507)
REM0 = 508                  # host-handled remainder rows REM0..511


def _vband(g: int, in_row: int) -> int:
    """Vertical 3-tap replicate-pad weight of input row `in_row` for output
    row g (rows 0..511; only the top pad matters, bottom rows are host's)."""
    w = 0
    for d in (-1, 0, 1):
        t = g + d
        if t < 0:
            t = 0
        if t == in_row:
            w += 1
    return w


def _build_bands() -> np.ndarray:
    """bands[P*4 + blk*2 + kind] = [128, 2*127] fp8 lhsT (kind 0=center with
    the 9*delta fold, 1=side)."""
    out = np.zeros((8, 128, 2, 128), np.float32)
    for P in range(2):
        base = PAIR_IN0[P]
        for blk in range(2):
            for m in range(127):
                g = 254 * P + 127 * blk + m
                for k in range(2):
                    for p in range(128):
                        in_row = base + 128 * k + p
                        v = _vband(g, in_row)
                        out[P * 4 + blk * 2 + 0, p, k, m] = (
                            9.0 * (in_row == g) - v
                        )
                        out[P * 4 + blk * 2 + 1, p, k, m] = -v
    # device layout: [p, (i, k, m)] -> [128, 2048]
    return (out.transpose(1, 0, 2, 3).reshape(128, 2048)).astype(NP_FP8)


def _build_module(bpc: int = BPC):
    nc = bacc.Bacc("TRN2")
    f32 = mybir.dt.float32
    bf16 = mybir.dt.bfloat16
    fp8 = mybir.dt.float8e4
    Alu = mybir.AluOpType
    Act = mybir.ActivationFunctionType
    DR = mybir.MatmulPerfMode.DoubleRow

    st = 2 * bpc
    xdev = nc.dram_tensor("xdev", [st, 128, 4 * WP], fp8, kind="ExternalInput")
    bands = nc.dram_tensor("bands", [128, 2048], fp8, kind="ExternalInput")
    outd = nc.dram_tensor("out", [6, 97, W], f32, kind="ExternalOutput")

    with tile.TileContext(nc) as tc, ExitStack() as ctx:
        const = ctx.enter_context(tc.tile_pool(name="const", bufs=1))
        xp = ctx.enter_context(tc.tile_pool(name="xp", bufs=4))
        bpools = [
            ctx.enter_context(tc.tile_pool(name=f"b{i}", bufs=2)) for i in range(4)
        ]
        qpools = [
            ctx.enter_context(tc.tile_pool(name=f"q{i}", bufs=2)) for i in range(2)
        ]
        sqp = ctx.enter_context(tc.tile_pool(name="sq", bufs=3))
        rsp = ctx.enter_context(tc.tile_pool(name="rs", bufs=4))
        mp = ctx.enter_context(tc.tile_pool(name="mp", bufs=6))
        evp = ctx.enter_context(tc.tile_pool(name="evp", bufs=6))
        pv = ctx.enter_context(tc.tile_pool(name="pv", bufs=5, space="PSUM"))
        pcs = ctx.enter_context(tc.tile_pool(name="pcs", bufs=1, space="PSUM"))

        bandt = const.tile([128, 16, 128], fp8, name="bandt", tag="bandt")
        nc.sync.dma_start(out=bandt[:], in_=bands[:])

        cs_tiles = {}

        def evac_round(g4):
            for qi in range(3):
                bank = 2 * qi + g4
                ev = evp.tile([97, W], f32, name=f"ev{bank}", tag=f"ev{bank}")
                if qi == 0:
                    nc.scalar.copy(ev[:], cs_tiles[qi][0:97, :])
                else:
                    nc.vector.tensor_copy(ev[:], cs_tiles[qi][0:97, :])
                nc.sync.dma_start(out=outd[bank], in_=ev[:])

        for s in range(bpc):
            if s % 4 == 0:
                for qi in range(3):
                    cs_tiles[qi] = pcs.tile([128, W], f32, name=f"cs{qi}",
                                            tag=f"cs{qi}")
                    nc.vector.memset(cs_tiles[qi][:], 0.0)
            Xt = []
            for t in range(2):
                X = xp.tile([128, 4, WP], fp8, tag=f"x{t}")
                nc.sync.dma_start(out=X[:, 0:2, :],
                                  in_=xdev[2 * s + t, :, 0 : 2 * WP])
                nc.sync.dma_start(out=X[:, 2:4, :],
                                  in_=xdev[2 * s + t, :, 2 * WP : 4 * WP])
                Xt.append(X)

            rst = [
                rsp.tile([128, 4], f32, name=f"rs{t}", tag=f"rs{t}")
                for t in range(2)
            ]
            btiles = {}
            qtiles = []
            for P in range(2):
                for t in range(2):
                    btile = bpools[2 * t + P].tile([128, 2, W], bf16, tag="b")
                    btiles[(t, P)] = btile
                    for blk in range(2):
                        u = pv.tile([128, W], f32, tag="u")
                        i8 = P * 4 + blk * 2
                        Cc = bandt[:, 2 * i8 : 2 * i8 + 2, :]
                        Cs = bandt[:, 2 * i8 + 2 : 2 * i8 + 4, :]
                        rhs = Xt[t][:, 2 * P : 2 * P + 2, :]
                        nc.tensor.matmul(u[:], Cc, rhs[:, :, 1 : W + 1],
                                         start=True, stop=False, perf_mode=DR)
                        nc.tensor.matmul(u[:], Cs, rhs[:, :, 2 : W + 2],
                                         start=False, stop=False, perf_mode=DR)
                        nc.tensor.matmul(u[:], Cs, rhs[:, :, 0:W],
                                         start=False, stop=True, perf_mode=DR)

                        unit = (2 * s + t) * 4 + P * 2 + blk
                        col = 2 * P + blk
                        bout = btile[:, blk, :]
                        racc = rst[t][:, col : col + 1]
                        if unit % ACT_MOD < ACT_LT:
                            sq = sqp.tile([128, W], bf16, tag="sq")
                            nc.scalar.activation(sq[:], u[:], Act.Square)
                            nc.scalar.activation(bout, sq[:], Act.Sign,
                                                 accum_out=racc)
                        else:
                            nc.vector.tensor_scalar(bout, u[:], 0.0, None,
                                                    Alu.not_equal, Alu.add,
                                                    accum_out=racc)
                # q for this pair
                qtile = qpools[P].tile([128, 2, W], bf16, tag="q")
                qtiles.append(qtile)
                qunit = 2 * s + P
                eng = nc.gpsimd if qunit % GPS_MOD < GPS_LT else nc.vector
                eng.tensor_tensor(qtile[:, :, :], btiles[(0, P)][:, :, :],
                                  btiles[(1, P)][:, :, :], Alu.mult)

            # row masks (tiny) on gpsimd
            ms = []
            for t in range(2):
                m = mp.tile([128, 4], bf16, tag=f"m{t}")
                nc.gpsimd.tensor_scalar(m[:], rst[t][:], LINE_T, None, Alu.is_lt)
                ms.append(m)
            mq = mp.tile([128, 4], bf16, tag="mq")
            nc.gpsimd.tensor_tensor(mq[:], ms[0][:], ms[1][:], Alu.mult)

            # masked column sums: 3 quantities x 2 pairs, DoubleRow M=1
            part = 32 * (s % 4)
            g4 = s // 4
            plan = [
                (ms[0], [btiles[(0, 0)], btiles[(0, 1)]]),
                (ms[1], [btiles[(1, 0)], btiles[(1, 1)]]),
                (mq, qtiles),
            ]
            for qi, (mask, rhs_pair) in enumerate(plan):
                cst = cs_tiles[qi]
                for P in range(2):
                    for blk in range(2):
                        col = 2 * P + blk
                        nc.tensor.matmul(cst[part : part + 1, :],
                                         mask[:, col : col + 1],
                                         rhs_pair[P][:, blk, :],
                                         start=(col == 0), stop=(col == 3),
                                         tile_position=(0, part))
            if s % 4 == 3:
                evac_round(s // 4)

        if bpc % 4 != 0:
            evac_round(bpc // 4)

    nc.compile()
    return nc


# ---------------------------------------------------------------- host side

_ROWMAP = np.concatenate(
    [np.arange(0, 128), np.arange(128, 256), np.arange(253, 381),
     np.arange(381, 509)]
)

_CACHE: dict = {}


def _get_module():
    if "nc" not in _CACHE:
        _CACHE["nc"] = _build_module()
        _CACHE["bands"] = _build_bands()
    return _CACHE["nc"], _CACHE["bands"]


def _pack_core(seg8: np.ndarray, gt8: np.ndarray) -> np.ndarray:
    """[n,512,512] f32 x2 -> xdev [2n, 128, 4*514] fp8 (ktile layout with
    replicate-padded edge columns)."""
    n = seg8.shape[0]
    xdev = np.empty((2 * n, 128, 4 * WP), NP_FP8)
    for s in range(n):
        for t, arr in ((0, seg8), (1, gt8)):
            v = np.pad(arr[s], ((0, 0), (1, 1)), mode="edge")  # [512, 514]
            v = v[_ROWMAP]
            v = v.reshape(4, 128, WP).transpose(1, 0, 2).reshape(128, 4 * WP)
            xdev[2 * s + t] = v.astype(NP_FP8)
    return xdev


def _host_rem(x: np.ndarray):
    """Boundary rows REM0..511 for all samples of one tensor.
    x: [B_, H, W] float {0,1}.  Returns (b [B_,4,W] f64, m [B_,4] f64)."""
    sub = x[:, REM0 - 1 : H, :].astype(np.float64)    # rows 507..511
    hp = np.pad(sub, ((0, 0), (0, 0), (1, 1)), mode="edge")
    h3 = hp[:, :, 0:W] + hp[:, :, 1 : W + 1] + hp[:, :, 2 : W + 2]
    w = np.empty((x.shape[0], 4, W))
    for i in range(4):
        # out row 508+i uses rows {507+i, 508+i, min(509+i, 511)} (replicate)
        lo, mid, hi = i, i + 1, min(i + 2, 4)
        w[:, i] = h3[:, lo] + h3[:, mid] + h3[:, hi]
    b = ((w > 0.5) & (w < 8.5)).astype(np.float64)
    m = (b.sum(axis=2) < LINE_T).astype(np.float64)
    return b, m


def _finish(outs: list[np.ndarray], seg: np.ndarray, gt: np.ndarray) -> np.ndarray:
    """outs: per-core 'out' arrays [6, 128, 512] f32."""
    bs_rem, ms_rem = _host_rem(seg)
    bg_rem, mg_rem = _host_rem(gt)
    total = 0.0
    for c, res in enumerate(outs):
        for s in range(BPC):
            part = 32 * (s % 4)
            g4 = s // 4
            gs = c * BPC + s
            cs_s = res[0 + g4, part].astype(np.float64)
            cs_g = res[2 + g4, part].astype(np.float64)
            Pv = res[4 + g4, part].astype(np.float64)
            cs_s = cs_s + (ms_rem[gs, :, None] * bs_rem[gs]).sum(axis=0)
            cs_g = cs_g + (mg_rem[gs, :, None] * bg_rem[gs]).sum(axis=0)
            Pv = Pv + (
                (ms_rem[gs] * mg_rem[gs])[:, None] * bs_rem[gs] * bg_rem[gs]
            ).sum(axis=0)
            ok_s = (cs_s < LINE_T).astype(np.float64)
            ok_g = (cs_g < LINE_T).astype(np.float64)
            total += float(
                np.sum(cs_s * ok_s) + np.sum(cs_g * ok_g)
                - 2.0 * np.sum(Pv * ok_s * ok_g)
            )
    return np.asarray(np.float32(4.0 * total / float(B * H * W)))


def make_in_maps(seg: np.ndarray, gt: np.ndarray, bands: np.ndarray):
    seg = np.ascontiguousarray(seg, dtype=np.float32)
    gt = np.ascontiguousarray(gt, dtype=np.float32)
    return [
        {
            "xdev": _pack_core(seg[c * BPC : (c + 1) * BPC],
                               gt[c * BPC : (c + 1) * BPC]),
            "bands": bands,
        }
        for c in range(N_CORES)
    ]


def kernel(seg: np.ndarray, gt: np.ndarray) -> np.ndarray:
    nc, bands = _get_module()
    in_maps = make_in_maps(seg, gt, bands)
    r = bass_utils.run_bass_kernel_spmd(nc, in_maps, core_ids=list(range(N_CORES)))
    return _finish([r.results[c]["out"] for c in range(N_CORES)], seg, gt)


# revision 26
# speedup vs baseline: 1.3145x; 1.1395x over previous
"""Trainium2 Bass kernel for nn_BoundaryConsistencyLoss (v3).

Math
----
Inputs seg/gt are binary {0,1} images [64, 512, 512].  For binary x the
per-class boundary (dilation - erosion with -inf padding) is the same for
both classes: b[i,j] = 1 iff the in-bounds 3x3 window at (i,j) is
non-constant.  The loss reduces to 4 * mean(xor(L(b_s), L(b_g))) with L the
row-then-column line-removal operator; the xor-count identity lets the host
finish from per-sample masked column sums (all exact small integers).

Let wsum = 3x3 replicate-pad window sum (total weight 9 everywhere) and
u = 9*x - wsum in [-8, 8].  Then b = (u != 0) -- a ONE-SIDED test, so the
fused DVE op  tensor_scalar(not_equal, accum=rowsum)  produces b and its
row sums in a single pass, and ACT can alternatively produce it with the
exact pair Square -> Sign(+accum).  u is built entirely on the PE with fp8
DoubleRow matmuls (K=256): 3 shifted band matmuls (C_center = 9*delta -
vband folds the 9x term into the center tap) + 2 single-column edge fixups.

Device per (sample, tensor): rows 0..507 as 4 blocks of 127 rows; the last
4 rows (508..511) are finished on the host (<1% of pixels).  Per sample the
device emits masked column sums cs_s, cs_g, P = sum_r m_s m_g b_s b_g via
fp8 DoubleRow matmuls with the row-mask vectors as stationary operands,
accumulated in PSUM and DMA'd straight to DRAM (no evacuation pass).

Sharding: pure data parallel over batch, 8 samples per NeuronCore.
"""

from contextlib import ExitStack

import ml_dtypes
import numpy as np

import concourse.bacc as bacc
import concourse.mybir as mybir
import concourse.tile as tile
from concourse import bass_utils

# ---------------------------------------------------------------- config
B, H, W = 64, 512, 512
WP = W + 2              # width with replicate-padded edge columns
N_CORES = 8
BPC = B // N_CORES          # samples per core
ST = 2 * BPC                # sample-tensors per core
LINE_T = 300.0

NP_FP8 = ml_dtypes.float8_e4m3fn

# engine split knobs
ACT_MOD, ACT_LT = 16, 6     # threshold unit -> ACT if (unit % MOD) < LT
GPS_MOD, GPS_LT = 8, 2      # q unit -> GpSimd if (qunit % MOD) < LT

# row-block geometry: pair P covers input rows [253P, 253P+255] as two
# DoubleRow k-tiles; block (P, blk) outputs rows 254P + 127*blk .. +126.
PAIR_IN0 = (0, 253)
NBLK = 4                    # blocks per sample-tensor (rows 0..507)
REM0 = 508                  # host-handled remainder rows REM0..511


def _vband(g: int, in_row: int) -> int:
    """Vertical 3-tap replicate-pad weight of input row `in_row` for output
    row g (rows 0..511; only the top pad matters, bottom rows are host's)."""
    w = 0
    for d in (-1, 0, 1):
        t = g + d
        if t < 0:
            t = 0
        if t == in_row:
            w += 1
    return w


def _build_bands() -> np.ndarray:
    """bands[P*4 + blk*2 + kind] = [128, 2*127] fp8 lhsT (kind 0=center with
    the 9*delta fold, 1=side)."""
    out = np.zeros((8, 128, 2, 128), np.float32)
    for P in range(2):
        base = PAIR_IN0[P]
        for blk in range(2):
            for m in range(127):
                g = 254 * P + 127 * blk + m
                for k in range(2):
                    for p in range(128):
                        in_row = base + 128 * k + p
                        v = _vband(g, in_row)
                        out[P * 4 + blk * 2 + 0, p, k, m] = (
                            9.0 * (in_row == g) - v
                        )
                        out[P * 4 + blk * 2 + 1, p, k, m] = -v
    # device layout: [p, (i, k, m)] -> [128, 2048]
    return (out.transpose(1, 0, 2, 3).reshape(128, 2048)).astype(NP_FP8)


def _build_module(bpc: int = BPC):
    nc = bacc.Bacc("TRN2")
    f32 = mybir.dt.float32
    bf16 = mybir.dt.bfloat16
    fp8 = mybir.dt.float8e4
    Alu = mybir.AluOpType
    Act = mybir.ActivationFunctionType
    DR = mybir.MatmulPerfMode.DoubleRow

    st = 2 * bpc
    xdev = nc.dram_tensor("xdev", [st, 128, 4 * WP], fp8, kind="ExternalInput")
    bands = nc.dram_tensor("bands", [128, 2048], fp8, kind="ExternalInput")
    outd = nc.dram_tensor("out", [6, 97, W], f32, kind="ExternalOutput")

    with tile.TileContext(nc) as tc, ExitStack() as ctx:
        const = ctx.enter_context(tc.tile_pool(name="const", bufs=1))
        xp = ctx.enter_context(tc.tile_pool(name="xp", bufs=1))
        bpools = [
            ctx.enter_context(tc.tile_pool(name=f"b{i}", bufs=2)) for i in range(4)
        ]
        qpools = [
            ctx.enter_context(tc.tile_pool(name=f"q{i}", bufs=2)) for i in range(2)
        ]
        sqp = ctx.enter_context(tc.tile_pool(name="sq", bufs=3))
        rsp = ctx.enter_context(tc.tile_pool(name="rs", bufs=4))
        mp = ctx.enter_context(tc.tile_pool(name="mp", bufs=6))
        evp = ctx.enter_context(tc.tile_pool(name="evp", bufs=6))
        pv = ctx.enter_context(tc.tile_pool(name="pv", bufs=5, space="PSUM"))
        pcs = ctx.enter_context(tc.tile_pool(name="pcs", bufs=1, space="PSUM"))

        bandt = const.tile([128, 16, 128], fp8, name="bandt", tag="bandt")
        nc.sync.dma_start(out=bandt[:], in_=bands[:])

        cs_tiles = {}

        def evac_round(g4):
            for qi in range(3):
                bank = 2 * qi + g4
                ev = evp.tile([97, W], f32, name=f"ev{bank}", tag=f"ev{bank}")
                if qi == 0:
                    nc.scalar.copy(ev[:], cs_tiles[qi][0:97, :])
                else:
                    nc.vector.tensor_copy(ev[:], cs_tiles[qi][0:97, :])
                nc.sync.dma_start(out=outd[bank], in_=ev[:])

        # prefetch the whole input: all tiles dispatched up front so the
        # 16 hardware DMA queues run concurrently instead of trickling
        Xall = []
        for sti in range(st):
            X = xp.tile([128, 4, WP], fp8, name=f"xt{sti}", tag=f"xt{sti}")
            nc.sync.dma_start(out=X[:, 0:2, :], in_=xdev[sti, :, 0 : 2 * WP])
            nc.sync.dma_start(out=X[:, 2:4, :], in_=xdev[sti, :, 2 * WP : 4 * WP])
            Xall.append(X)

        for s in range(bpc):
            if s % 4 == 0:
                for qi in range(3):
                    cs_tiles[qi] = pcs.tile([128, W], f32, name=f"cs{qi}",
                                            tag=f"cs{qi}")
                    nc.vector.memset(cs_tiles[qi][:], 0.0)
            Xt = [Xall[2 * s], Xall[2 * s + 1]]

            rst = [
                rsp.tile([128, 4], f32, name=f"rs{t}", tag=f"rs{t}")
                for t in range(2)
            ]
            btiles = {}
            qtiles = []
            for P in range(2):
                for t in range(2):
                    btile = bpools[2 * t + P].tile([128, 2, W], bf16, tag="b")
                    btiles[(t, P)] = btile
                    for blk in range(2):
                        u = pv.tile([128, W], f32, tag="u")
                        i8 = P * 4 + blk * 2
                        Cc = bandt[:, 2 * i8 : 2 * i8 + 2, :]
                        Cs = bandt[:, 2 * i8 + 2 : 2 * i8 + 4, :]
                        rhs = Xt[t][:, 2 * P : 2 * P + 2, :]
                        nc.tensor.matmul(u[:], Cc, rhs[:, :, 1 : W + 1],
                                         start=True, stop=False, perf_mode=DR)
                        nc.tensor.matmul(u[:], Cs, rhs[:, :, 2 : W + 2],
                                         start=False, stop=False, perf_mode=DR)
                        nc.tensor.matmul(u[:], Cs, rhs[:, :, 0:W],
                                         start=False, stop=True, perf_mode=DR)

                        unit = (2 * s + t) * 4 + P * 2 + blk
                        col = 2 * P + blk
                        bout = btile[:, blk, :]
                        racc = rst[t][:, col : col + 1]
                        if unit % ACT_MOD < ACT_LT:
                            sq = sqp.tile([128, W], bf16, tag="sq")
                            nc.scalar.activation(sq[:], u[:], Act.Square)
                            nc.scalar.activation(bout, sq[:], Act.Sign,
                                                 accum_out=racc)
                        else:
                            nc.vector.tensor_scalar(bout, u[:], 0.0, None,
                                                    Alu.not_equal, Alu.add,
                                                    accum_out=racc)
                # q for this pair
                qtile = qpools[P].tile([128, 2, W], bf16, tag="q")
                qtiles.append(qtile)
                qunit = 2 * s + P
                eng = nc.gpsimd if qunit % GPS_MOD < GPS_LT else nc.vector
                eng.tensor_tensor(qtile[:, :, :], btiles[(0, P)][:, :, :],
                                  btiles[(1, P)][:, :, :], Alu.mult)

            # row masks (tiny) on gpsimd
            ms = []
            for t in range(2):
                m = mp.tile([128, 4], bf16, tag=f"m{t}")
                nc.gpsimd.tensor_scalar(m[:], rst[t][:], LINE_T, None, Alu.is_lt)
                ms.append(m)
            mq = mp.tile([128, 4], bf16, tag="mq")
            nc.gpsimd.tensor_tensor(mq[:], ms[0][:], ms[1][:], Alu.mult)

            # masked column sums: 3 quantities x 2 pairs, DoubleRow M=1
            part = 32 * (s % 4)
            g4 = s // 4
            plan = [
                (ms[0], [btiles[(0, 0)], btiles[(0, 1)]]),
                (ms[1], [btiles[(1, 0)], btiles[(1, 1)]]),
                (mq, qtiles),
            ]
            for qi, (mask, rhs_pair) in enumerate(plan):
                cst = cs_tiles[qi]
                for P in range(2):
                    for blk in range(2):
                        col = 2 * P + blk
                        nc.tensor.matmul(cst[part : part + 1, :],
                                         mask[:, col : col + 1],
                                         rhs_pair[P][:, blk, :],
                                         start=(col == 0), stop=(col == 3),
                                         tile_position=(0, part))
            if s % 4 == 3:
                evac_round(s // 4)

        if bpc % 4 != 0:
            evac_round(bpc // 4)

    nc.compile()
    return nc


# ---------------------------------------------------------------- host side

_ROWMAP = np.concatenate(
    [np.arange(0, 128), np.arange(128, 256), np.arange(253, 381),
     np.arange(381, 509)]
)

_CACHE: dict = {}


def _get_module():
    if "nc" not in _CACHE:
        _CACHE["nc"] = _build_module()
        _CACHE["bands"] = _build_bands()
    return _CACHE["nc"], _CACHE["bands"]


def _pack_core(seg8: np.ndarray, gt8: np.ndarray) -> np.ndarray:
    """[n,512,512] f32 x2 -> xdev [2n, 128, 4*514] fp8 (ktile layout with
    replicate-padded edge columns)."""
    n = seg8.shape[0]
    xdev = np.empty((2 * n, 128, 4 * WP), NP_FP8)
    for s in range(n):
        for t, arr in ((0, seg8), (1, gt8)):
            v = np.pad(arr[s], ((0, 0), (1, 1)), mode="edge")  # [512, 514]
            v = v[_ROWMAP]
            v = v.reshape(4, 128, WP).transpose(1, 0, 2).reshape(128, 4 * WP)
            xdev[2 * s + t] = v.astype(NP_FP8)
    return xdev


def _host_rem(x: np.ndarray):
    """Boundary rows REM0..511 for all samples of one tensor.
    x: [B_, H, W] float {0,1}.  Returns (b [B_,4,W] f64, m [B_,4] f64)."""
    sub = x[:, REM0 - 1 : H, :].astype(np.float64)    # rows 507..511
    hp = np.pad(sub, ((0, 0), (0, 0), (1, 1)), mode="edge")
    h3 = hp[:, :, 0:W] + hp[:, :, 1 : W + 1] + hp[:, :, 2 : W + 2]
    w = np.empty((x.shape[0], 4, W))
    for i in range(4):
        # out row 508+i uses rows {507+i, 508+i, min(509+i, 511)} (replicate)
        lo, mid, hi = i, i + 1, min(i + 2, 4)
        w[:, i] = h3[:, lo] + h3[:, mid] + h3[:, hi]
    b = ((w > 0.5) & (w < 8.5)).astype(np.float64)
    m = (b.sum(axis=2) < LINE_T).astype(np.float64)
    return b, m


def _finish(outs: list[np.ndarray], seg: np.ndarray, gt: np.ndarray) -> np.ndarray:
    """outs: per-core 'out' arrays [6, 128, 512] f32."""
    bs_rem, ms_rem = _host_rem(seg)
    bg_rem, mg_rem = _host_rem(gt)
    total = 0.0
    for c, res in enumerate(outs):
        for s in range(BPC):
            part = 32 * (s % 4)
            g4 = s // 4
            gs = c * BPC + s
            cs_s = res[0 + g4, part].astype(np.float64)
            cs_g = res[2 + g4, part].astype(np.float64)
            Pv = res[4 + g4, part].astype(np.float64)
            cs_s = cs_s + (ms_rem[gs, :, None] * bs_rem[gs]).sum(axis=0)
            cs_g = cs_g + (mg_rem[gs, :, None] * bg_rem[gs]).sum(axis=0)
            Pv = Pv + (
                (ms_rem[gs] * mg_rem[gs])[:, None] * bs_rem[gs] * bg_rem[gs]
            ).sum(axis=0)
            ok_s = (cs_s < LINE_T).astype(np.float64)
            ok_g = (cs_g < LINE_T).astype(np.float64)
            total += float(
                np.sum(cs_s * ok_s) + np.sum(cs_g * ok_g)
                - 2.0 * np.sum(Pv * ok_s * ok_g)
            )
    return np.asarray(np.float32(4.0 * total / float(B * H * W)))


def make_in_maps(seg: np.ndarray, gt: np.ndarray, bands: np.ndarray):
    seg = np.ascontiguousarray(seg, dtype=np.float32)
    gt = np.ascontiguousarray(gt, dtype=np.float32)
    return [
        {
            "xdev": _pack_core(seg[c * BPC : (c + 1) * BPC],
                               gt[c * BPC : (c + 1) * BPC]),
            "bands": bands,
        }
        for c in range(N_CORES)
    ]


def kernel(seg: np.ndarray, gt: np.ndarray) -> np.ndarray:
    nc, bands = _get_module()
    in_maps = make_in_maps(seg, gt, bands)
    r = bass_utils.run_bass_kernel_spmd(nc, in_maps, core_ids=list(range(N_CORES)))
    return _finish([r.results[c]["out"] for c in range(N_CORES)], seg, gt)
